# revision 19
# baseline (speedup 1.0000x reference)
"""Causal depthwise conv1d (K=4) over packed ragged sequences + SiLU + conv-state
cache update, sharded channel-wise across 8 trn2 NeuronCores.

Strategy:
  - Channels (D=4096) sharded 512/core (tensor-parallel, per the module's tp
    logic). Host transposes x to channel-major (D, T) so each core DMAs
    contiguous rows; on-chip layout is [channels->partitions, tokens->free],
    so conv taps are free-dim offsets.
  - HBM read efficiency needs long per-partition rows (2KB rows ~250GB/s vs
    32KB rows ~395GB/s with all 8 cores saturating device HBM), so x is
    loaded in [128, 8192] megatiles; writes are insensitive to chunk size, so
    each compute item stores its own output tile (keeps the pipeline
    fine-grained).
  - No single engine covers 4 fp32 taps under the HBM floor (~175us/core):
    fp32 PE matmul is 2-pass (~4cyc/col), fp32 DVE tensor-ops run at 1x.
    Token ranges are split across engines:
      * PE tiles (512 tok): 4 diagonal-matrix matmuls accumulate taps in
        PSUM (per-channel scale = diagonal stationary); ScalarE
        silu(psum+bias) into an output tile.
      * DVE quads (2048/1536 tok): ScalarE seeds tap0 (Copy with per-channel
        scale), VectorE chains 3 scalar_tensor_tensor fused MACs in place,
        ScalarE silu.
    ~34 PE tiles vs ~23 quads per core balances PE (~173us) against DVE
    (~165us).
  - Sequence-boundary tokens (first 3 of each sequence, <= 27 rows) are
    recomputed exactly on the host; the conv-state cache update (1MB
    gather/scatter) is metadata-sized and also done on the host.
"""

import numpy as np

T = 16384
D = 4096
K = 4
NCORES = 8
DC = D // NCORES  # 512 channels per core
G = DC // 128     # 4 partition groups per core
HALO = K - 1      # 3

F_IO = 8192       # input megatile tokens (32KB rows)
F_PE = 512        # PE tile (one fp32 PSUM bank)
F_DVE = 2048      # DVE quad

# Work items per megatile, as (kind, local_offset, width):
#   'A' half: 4 PE tiles + 3 quads (the plain half)
#   'E' half: 5 PE tiles + 2 quads-2048 + 1 quad-1536 (PE-heavier)
_A = [("dve", 2048, 2048), ("dve", 4096, 2048), ("dve", 6144, 2048),
      ("pe", 0, F_PE), ("pe", 512, F_PE), ("pe", 1024, F_PE),
      ("pe", 1536, F_PE)]
_E = [("dve", 2048, 2048), ("dve", 4096, 2048), ("dve", 6144, 1536),
      ("pe", 0, F_PE), ("pe", 512, F_PE), ("pe", 1024, F_PE),
      ("pe", 1536, F_PE), ("pe", 7680, F_PE)]
# even groups: 8 PE tiles; odd groups: 9 PE tiles  -> 34 PE tiles/core
GROUP_PLANS = {0: (_A, _A), 1: (_A, _E), 2: (_A, _A), 3: (_A, _E)}

_cached_nc = None


def _build_device_kernel():
    import concourse.bacc as bacc
    import concourse.mybir as mybir
    from concourse.masks import make_identity
    from concourse.tile import TileContext

    f32 = mybir.dt.float32
    mult = mybir.AluOpType.mult
    add = mybir.AluOpType.add
    silu_fn = mybir.ActivationFunctionType.Silu
    copy_fn = mybir.ActivationFunctionType.Copy

    nc = bacc.Bacc("TRN2", target_bir_lowering=False, debug=False,
                   num_devices=NCORES)

    xt = nc.dram_tensor("xt", [DC, T], f32, kind="ExternalInput")
    ws = nc.dram_tensor("ws", [128, G, K], f32, kind="ExternalInput")
    bs = nc.dram_tensor("bs", [128, G], f32, kind="ExternalInput")
    yt = nc.dram_tensor("yt", [DC, T], f32, kind="ExternalOutput")

    with TileContext(nc) as tc:
        with (
            tc.tile_pool(name="const", bufs=1) as cpool,
            tc.tile_pool(name="xb", bufs=3) as xpool,
            tc.tile_pool(name="ype", bufs=8) as ype_pool,
            tc.tile_pool(name="acc", bufs=4) as acc_pool,
            tc.tile_pool(name="ps", bufs=8, space="PSUM") as ppool,
        ):
            ws_sb = cpool.tile([128, G, K], f32)
            nc.sync.dma_start(out=ws_sb[:], in_=ws[:])
            bs_sb = cpool.tile([128, G], f32)
            nc.sync.dma_start(out=bs_sb[:], in_=bs[:])
            ident = cpool.tile([128, 128], f32)
            make_identity(nc, ident[:])
            wd_sb = cpool.tile([128, G, K, 128], f32)
            for g in range(G):
                for j in range(K):
                    nc.vector.tensor_scalar_mul(
                        wd_sb[:, g, j, :], ident[:], ws_sb[:, g, j:j + 1])

            for g in range(G):
                rows = slice(g * 128, (g + 1) * 128)
                for half, plan in enumerate(GROUP_PLANS[g]):
                    t0 = half * F_IO
                    xb = xpool.tile([128, F_IO + HALO], f32)
                    if t0 == 0:
                        nc.gpsimd.memset(xb[:, 0:HALO], 0.0)
                        nc.sync.dma_start(out=xb[:, HALO:], in_=xt[rows, 0:F_IO])
                    else:
                        nc.sync.dma_start(
                            out=xb[:], in_=xt[rows, t0 - HALO:t0 + F_IO])

                    for kind, u, width in plan:
                        if kind == "pe":
                            ps = ppool.tile([128, F_PE], f32)
                            for j in range(K):
                                # tap j: out[c,t] += w[c,j]*x[c, t-(K-1-j)]
                                nc.tensor.matmul(
                                    ps[:], wd_sb[:, g, j, :],
                                    xb[:, u + j:u + j + F_PE],
                                    start=(j == 0), stop=(j == K - 1),
                                )
                            yp = ype_pool.tile([128, F_PE], f32)
                            nc.scalar.activation(
                                yp[:], ps[:], silu_fn,
                                bias=bs_sb[:, g:g + 1], scale=1.0)
                            nc.sync.dma_start(
                                out=yt[rows, t0 + u:t0 + u + F_PE], in_=yp[:])
                        else:
                            acc = acc_pool.tile([128, F_DVE], f32)
                            av = acc[:, 0:width]
                            nc.scalar.activation(
                                av, xb[:, u:u + width], copy_fn,
                                bias=0.0, scale=ws_sb[:, g, 0:1])
                            for j in (1, 2, 3):
                                nc.vector.scalar_tensor_tensor(
                                    out=av, in0=xb[:, u + j:u + j + width],
                                    scalar=ws_sb[:, g, j:j + 1], in1=av,
                                    op0=mult, op1=add)
                            nc.scalar.activation(
                                av, av, silu_fn,
                                bias=bs_sb[:, g:g + 1], scale=1.0)
                            nc.sync.dma_start(
                                out=yt[rows, t0 + u:t0 + u + width], in_=av)

    nc.compile()
    return nc


def _get_nc():
    global _cached_nc
    if _cached_nc is None:
        _cached_nc = _build_device_kernel()
    return _cached_nc


def _silu(a):
    return a * (1.0 / (1.0 + np.exp(-a)))


def kernel(x, weight, bias, conv_state, seq_idx, conv_idx, state_ids,
           _run_opts=None):
    from concourse.bass_utils import run_bass_kernel_spmd

    x = np.asarray(x)
    weight = np.asarray(weight)
    bias = np.asarray(bias)
    conv_state = np.asarray(conv_state)
    seq_idx = np.asarray(seq_idx)
    conv_idx = np.asarray(conv_idx)
    state_ids = np.asarray(state_ids)

    x0 = x[0]                               # (T, D) f32
    w = weight[:, 0, :].astype(np.float32)  # (D, K)
    xT = np.ascontiguousarray(x0.T)         # (D, T)

    in_maps = []
    for c in range(NCORES):
        lo = c * DC
        w_core = w[lo:lo + DC]              # (DC, K)
        wsm = np.ascontiguousarray(
            w_core.reshape(G, 128, K).transpose(1, 0, 2))   # (128, G, K)
        bsm = np.ascontiguousarray(
            bias[lo:lo + DC].astype(np.float32).reshape(G, 128).T)  # (128, G)
        in_maps.append({
            "xt": np.ascontiguousarray(xT[lo:lo + DC]),
            "ws": wsm,
            "bs": bsm,
        })

    nc = _get_nc()
    run_opts = _run_opts or {}
    res = run_bass_kernel_spmd(nc, in_maps, core_ids=list(range(NCORES)),
                               **run_opts)

    outT = np.concatenate([r["yt"] for r in res.results], axis=0)  # (D, T)
    out = np.ascontiguousarray(outT.T)[None]                       # (1, T, D)

    # --- host fixup: first K-1 tokens of every sequence (exact recompute) ---
    starts = np.concatenate([[0], np.flatnonzero(np.diff(seq_idx) != 0) + 1])
    fix = (starts[:, None] + np.arange(HALO)[None]).ravel()
    fix = np.unique(fix[fix < T])
    if fix.size:
        acc = np.broadcast_to(bias.astype(np.float32), (fix.size, D)).copy()
        for j in range(K):
            s = K - 1 - j
            tm = fix - s
            tm_c = np.clip(tm, 0, T - 1)
            valid = (tm >= 0) & (seq_idx[tm_c] == seq_idx[fix])
            acc += np.where(valid[:, None], x0[tm_c], 0.0) * w[None, :, j]
        out[0, fix] = _silu(acc)

    # --- conv-state cache update (gather last-K rows, scatter into pool) ---
    new_conv_state = conv_state.copy()
    new_conv_state[state_ids] = np.transpose(x0[conv_idx], (0, 2, 1))

    if _run_opts is not None:
        return (out, new_conv_state), res
    return out, new_conv_state


# revision 21
# speedup vs baseline: 1.0034x; 1.0034x over previous
"""Causal depthwise conv1d (K=4) over packed ragged sequences + SiLU + conv-state
cache update, sharded channel-wise across 8 trn2 NeuronCores.

Strategy:
  - Channels (D=4096) sharded 512/core (tensor-parallel, per the module's tp
    logic). Host transposes x to channel-major (D, T) so each core DMAs
    contiguous rows; on-chip layout is [channels->partitions, tokens->free],
    so conv taps are free-dim offsets.
  - HBM reads need long per-partition rows (2KB rows ~250GB/s vs 32KB rows
    ~395GB/s with all 8 cores saturating device HBM), so x is loaded in
    [128, 8192] megatiles; writes are insensitive to chunk size, so each
    compute item stores its own output tile.
  - No single engine covers 4 fp32 taps under the HBM floor (~175us/core):
    fp32 PE matmul is 2-pass (~4cyc/col) and fp32 DVE tensor-ops run at 1x.
    All work is 2048-token quads of two kinds, interleaved 1:1:
      * S1: PE computes taps 0-2 as diagonal-matrix matmuls into a 4-bank
        PSUM tile (per-channel scale = diagonal stationary); VectorE adds
        tap 3 with one scalar_tensor_tensor whose in1 reads the PSUM
        partial; ScalarE silu(acc+bias).
      * S2: ScalarE seeds tap 0 (Copy with per-channel scale), VectorE
        chains 3 scalar_tensor_tensor fused MACs, ScalarE silu.
    -> PE ~160us, DVE ~150us, ACT ~95us, under the DMA floor.
  - Sequence-boundary tokens (first 3 of each sequence, <= 27 rows) are
    recomputed exactly on the host; the conv-state cache update (1MB
    gather/scatter) is metadata-sized and also done on the host.
"""

import numpy as np

T = 16384
D = 4096
K = 4
NCORES = 8
DC = D // NCORES  # 512 channels per core
G = DC // 128     # 4 partition groups per core
HALO = K - 1      # 3

F_IO = 8192       # input megatile tokens (32KB rows)
F_Q = 2048        # quad tokens (4 PSUM banks for S1)
NBANK = F_Q // 512
# quad types per megatile (4 quads each), per (group, half):
QUAD_PLAN = ("s1", "s2", "s1", "s2")

_cached_nc = None


def _build_device_kernel():
    import concourse.bacc as bacc
    import concourse.mybir as mybir
    from concourse.masks import make_identity
    from concourse.tile import TileContext

    f32 = mybir.dt.float32
    mult = mybir.AluOpType.mult
    add = mybir.AluOpType.add
    silu_fn = mybir.ActivationFunctionType.Silu
    copy_fn = mybir.ActivationFunctionType.Copy

    nc = bacc.Bacc("TRN2", target_bir_lowering=False, debug=False,
                   num_devices=NCORES)

    xt = nc.dram_tensor("xt", [DC, T], f32, kind="ExternalInput")
    ws = nc.dram_tensor("ws", [128, G, K], f32, kind="ExternalInput")
    bs = nc.dram_tensor("bs", [128, G], f32, kind="ExternalInput")
    yt = nc.dram_tensor("yt", [DC, T], f32, kind="ExternalOutput")

    with TileContext(nc) as tc:
        with (
            tc.tile_pool(name="const", bufs=1) as cpool,
            tc.tile_pool(name="xb", bufs=3) as xpool,
            tc.tile_pool(name="acc", bufs=6) as acc_pool,
            tc.tile_pool(name="ps", bufs=2, space="PSUM") as ppool,
        ):
            ws_sb = cpool.tile([128, G, K], f32)
            nc.sync.dma_start(out=ws_sb[:], in_=ws[:])
            bs_sb = cpool.tile([128, G], f32)
            nc.sync.dma_start(out=bs_sb[:], in_=bs[:])
            ident = cpool.tile([128, 128], f32)
            make_identity(nc, ident[:])
            wd_sb = cpool.tile([128, G, K, 128], f32)
            for g in range(G):
                for j in range(K):
                    nc.vector.tensor_scalar_mul(
                        wd_sb[:, g, j, :], ident[:], ws_sb[:, g, j:j + 1])

            for g in range(G):
                rows = slice(g * 128, (g + 1) * 128)
                for half in range(T // F_IO):
                    t0 = half * F_IO
                    xb = xpool.tile([128, F_IO + HALO], f32)
                    if t0 == 0:
                        nc.gpsimd.memset(xb[:, 0:HALO], 0.0)
                        nc.sync.dma_start(out=xb[:, HALO:], in_=xt[rows, 0:F_IO])
                    else:
                        nc.sync.dma_start(
                            out=xb[:], in_=xt[rows, t0 - HALO:t0 + F_IO])

                    for q, typ in enumerate(QUAD_PLAN):
                        u = q * F_Q
                        acc = acc_pool.tile([128, F_Q], f32)
                        if typ == "s1":
                            ps = ppool.tile([128, F_Q], f32)
                            for j in range(K - 1):
                                for b in range(NBANK):
                                    ub = u + b * 512
                                    nc.tensor.matmul(
                                        ps[:, b * 512:(b + 1) * 512],
                                        wd_sb[:, g, j, :],
                                        xb[:, ub + j:ub + j + 512],
                                        start=(j == 0), stop=(j == K - 2),
                                    )
                            # tap 3 rides the PSUM partial via in1
                            # (one op per bank: engines read one PSUM bank
                            # per instruction)
                            for b in range(NBANK):
                                lo_, hi_ = b * 512, (b + 1) * 512
                                nc.vector.scalar_tensor_tensor(
                                    out=acc[:, lo_:hi_],
                                    in0=xb[:, u + 3 + lo_:u + 3 + hi_],
                                    scalar=ws_sb[:, g, 3:4],
                                    in1=ps[:, lo_:hi_],
                                    op0=mult, op1=add)
                        else:
                            nc.scalar.activation(
                                acc[:], xb[:, u:u + F_Q], copy_fn,
                                bias=0.0, scale=ws_sb[:, g, 0:1])
                            for j in (1, 2, 3):
                                nc.vector.scalar_tensor_tensor(
                                    out=acc[:], in0=xb[:, u + j:u + j + F_Q],
                                    scalar=ws_sb[:, g, j:j + 1], in1=acc[:],
                                    op0=mult, op1=add)
                        nc.scalar.activation(
                            acc[:], acc[:], silu_fn,
                            bias=bs_sb[:, g:g + 1], scale=1.0)
                        nc.sync.dma_start(
                            out=yt[rows, t0 + u:t0 + u + F_Q], in_=acc[:])

    nc.compile()
    return nc


def _get_nc():
    global _cached_nc
    if _cached_nc is None:
        _cached_nc = _build_device_kernel()
    return _cached_nc


def _silu(a):
    return a * (1.0 / (1.0 + np.exp(-a)))


def kernel(x, weight, bias, conv_state, seq_idx, conv_idx, state_ids,
           _run_opts=None):
    from concourse.bass_utils import run_bass_kernel_spmd

    x = np.asarray(x)
    weight = np.asarray(weight)
    bias = np.asarray(bias)
    conv_state = np.asarray(conv_state)
    seq_idx = np.asarray(seq_idx)
    conv_idx = np.asarray(conv_idx)
    state_ids = np.asarray(state_ids)

    x0 = x[0]                               # (T, D) f32
    w = weight[:, 0, :].astype(np.float32)  # (D, K)
    xT = np.ascontiguousarray(x0.T)         # (D, T)

    in_maps = []
    for c in range(NCORES):
        lo = c * DC
        w_core = w[lo:lo + DC]              # (DC, K)
        wsm = np.ascontiguousarray(
            w_core.reshape(G, 128, K).transpose(1, 0, 2))   # (128, G, K)
        bsm = np.ascontiguousarray(
            bias[lo:lo + DC].astype(np.float32).reshape(G, 128).T)  # (128, G)
        in_maps.append({
            "xt": np.ascontiguousarray(xT[lo:lo + DC]),
            "ws": wsm,
            "bs": bsm,
        })

    nc = _get_nc()
    run_opts = _run_opts or {}
    res = run_bass_kernel_spmd(nc, in_maps, core_ids=list(range(NCORES)),
                               **run_opts)

    outT = np.concatenate([r["yt"] for r in res.results], axis=0)  # (D, T)
    out = np.ascontiguousarray(outT.T)[None]                       # (1, T, D)

    # --- host fixup: first K-1 tokens of every sequence (exact recompute) ---
    starts = np.concatenate([[0], np.flatnonzero(np.diff(seq_idx) != 0) + 1])
    fix = (starts[:, None] + np.arange(HALO)[None]).ravel()
    fix = np.unique(fix[fix < T])
    if fix.size:
        acc = np.broadcast_to(bias.astype(np.float32), (fix.size, D)).copy()
        for j in range(K):
            s = K - 1 - j
            tm = fix - s
            tm_c = np.clip(tm, 0, T - 1)
            valid = (tm >= 0) & (seq_idx[tm_c] == seq_idx[fix])
            acc += np.where(valid[:, None], x0[tm_c], 0.0) * w[None, :, j]
        out[0, fix] = _silu(acc)

    # --- conv-state cache update (gather last-K rows, scatter into pool) ---
    new_conv_state = conv_state.copy()
    new_conv_state[state_ids] = np.transpose(x0[conv_idx], (0, 2, 1))

    if _run_opts is not None:
        return (out, new_conv_state), res
    return out, new_conv_state


# revision 22
# speedup vs baseline: 1.0116x; 1.0081x over previous
"""Causal depthwise conv1d (K=4) over packed ragged sequences + SiLU + conv-state
cache update, sharded channel-wise across 8 trn2 NeuronCores.

Strategy:
  - Channels (D=4096) sharded 512/core (tensor-parallel, per the module's tp
    logic). Host transposes x to channel-major (D, T) so each core DMAs
    contiguous rows; on-chip layout is [channels->partitions, tokens->free],
    so conv taps are free-dim offsets.
  - HBM reads need long per-partition rows (2KB rows ~250GB/s vs 32KB rows
    ~395GB/s with all 8 cores saturating device HBM), so x is loaded in
    [128, 8192] megatiles; writes are insensitive to chunk size, so each
    compute item stores its own output tile (keeps the pipeline
    fine-grained).
  - No single engine covers 4 fp32 taps under the HBM floor (~175us/core):
    fp32 PE matmul is 2-pass (~2.4cyc/col sustained) and fp32 DVE
    tensor-tensor ops run at 1 elem/lane/cyc. Token ranges are split:
      * PE tiles (512 tok): 4 diagonal-matrix matmuls accumulate taps in
        PSUM (per-channel scale = diagonal stationary); ScalarE
        silu(psum+bias).
      * DVE quads (2048/1536 tok): ScalarE seeds tap0 (Copy with per-channel
        scale), VectorE chains 3 scalar_tensor_tensor fused MACs in place,
        ScalarE silu.
    ~34 PE tiles vs ~23 quads per core balances PE (~170us) against DVE
    (~165us), matching the min-max assignment over (PE 1.24us, DVE 0.59us,
    ACT-seed 0.61us) per 512-token tap unit.
  - Sequence-boundary tokens (first 3 of each sequence, <= 27 rows) are
    recomputed exactly on the host; the conv-state cache update (1MB
    gather/scatter) is metadata-sized and also done on the host.
"""

import numpy as np

T = 16384
D = 4096
K = 4
NCORES = 8
DC = D // NCORES  # 512 channels per core
G = DC // 128     # 4 partition groups per core
HALO = K - 1      # 3

F_IO = 8192       # input megatile tokens (32KB rows)
F_PE = 512        # PE tile (one fp32 PSUM bank)
F_DVE = 2048      # DVE quad

# Work items per megatile, as (kind, local_offset, width):
#   'A' half: 4 PE tiles + 3 quads (the plain half)
#   'E' half: 5 PE tiles + 2 quads-2048 + 1 quad-1536 (PE-heavier)
_A = [("dve", 2048, 2048), ("dve", 4096, 2048), ("dve", 6144, 2048),
      ("pe", 0, F_PE), ("pe", 512, F_PE), ("pe", 1024, F_PE),
      ("pe", 1536, F_PE)]
_E = [("dve", 2048, 2048), ("dve", 4096, 2048), ("dve", 6144, 1536),
      ("pe", 0, F_PE), ("pe", 512, F_PE), ("pe", 1024, F_PE),
      ("pe", 1536, F_PE), ("pe", 7680, F_PE)]
# even groups: 8 PE tiles; odd groups: 9 PE tiles  -> 34 PE tiles/core
GROUP_PLANS = {0: (_A, _A), 1: (_A, _E), 2: (_A, _A), 3: (_A, _E)}

_cached_nc = None


def _build_device_kernel():
    import concourse.bacc as bacc
    import concourse.mybir as mybir
    from concourse.masks import make_identity
    from concourse.tile import TileContext

    f32 = mybir.dt.float32
    mult = mybir.AluOpType.mult
    add = mybir.AluOpType.add
    silu_fn = mybir.ActivationFunctionType.Silu
    copy_fn = mybir.ActivationFunctionType.Copy

    nc = bacc.Bacc("TRN2", target_bir_lowering=False, debug=False,
                   num_devices=NCORES)

    xt = nc.dram_tensor("xt", [DC, T], f32, kind="ExternalInput")
    ws = nc.dram_tensor("ws", [128, G, K], f32, kind="ExternalInput")
    bs = nc.dram_tensor("bs", [128, G], f32, kind="ExternalInput")
    yt = nc.dram_tensor("yt", [DC, T], f32, kind="ExternalOutput")

    with TileContext(nc) as tc:
        with (
            tc.tile_pool(name="const", bufs=1) as cpool,
            tc.tile_pool(name="xb", bufs=4) as xpool,
            tc.tile_pool(name="ype", bufs=6) as ype_pool,
            tc.tile_pool(name="acc", bufs=4) as acc_pool,
            tc.tile_pool(name="ps", bufs=8, space="PSUM") as ppool,
        ):
            ws_sb = cpool.tile([128, G, K], f32)
            nc.sync.dma_start(out=ws_sb[:], in_=ws[:])
            bs_sb = cpool.tile([128, G], f32)
            nc.sync.dma_start(out=bs_sb[:], in_=bs[:])
            ident = cpool.tile([128, 128], f32)
            make_identity(nc, ident[:])
            wd_sb = cpool.tile([128, G, K, 128], f32)
            for g in range(G):
                for j in range(K):
                    nc.vector.tensor_scalar_mul(
                        wd_sb[:, g, j, :], ident[:], ws_sb[:, g, j:j + 1])

            for g in range(G):
                rows = slice(g * 128, (g + 1) * 128)
                for half, plan in enumerate(GROUP_PLANS[g]):
                    t0 = half * F_IO
                    xb = xpool.tile([128, F_IO + HALO], f32)
                    if t0 == 0:
                        nc.gpsimd.memset(xb[:, 0:HALO], 0.0)
                        nc.sync.dma_start(out=xb[:, HALO:], in_=xt[rows, 0:F_IO])
                    else:
                        nc.sync.dma_start(
                            out=xb[:], in_=xt[rows, t0 - HALO:t0 + F_IO])

                    for kind, u, width in plan:
                        if kind == "pe":
                            ps = ppool.tile([128, F_PE], f32)
                            for j in range(K):
                                # tap j: out[c,t] += w[c,j]*x[c, t-(K-1-j)]
                                nc.tensor.matmul(
                                    ps[:], wd_sb[:, g, j, :],
                                    xb[:, u + j:u + j + F_PE],
                                    start=(j == 0), stop=(j == K - 1),
                                )
                            yp = ype_pool.tile([128, F_PE], f32)
                            nc.scalar.activation(
                                yp[:], ps[:], silu_fn,
                                bias=bs_sb[:, g:g + 1], scale=1.0)
                            nc.sync.dma_start(
                                out=yt[rows, t0 + u:t0 + u + F_PE], in_=yp[:])
                        else:
                            acc = acc_pool.tile([128, F_DVE], f32)
                            av = acc[:, 0:width]
                            nc.scalar.activation(
                                av, xb[:, u:u + width], copy_fn,
                                bias=0.0, scale=ws_sb[:, g, 0:1])
                            for j in (1, 2, 3):
                                nc.vector.scalar_tensor_tensor(
                                    out=av, in0=xb[:, u + j:u + j + width],
                                    scalar=ws_sb[:, g, j:j + 1], in1=av,
                                    op0=mult, op1=add)
                            nc.scalar.activation(
                                av, av, silu_fn,
                                bias=bs_sb[:, g:g + 1], scale=1.0)
                            nc.sync.dma_start(
                                out=yt[rows, t0 + u:t0 + u + width], in_=av)

    nc.compile()
    return nc


def _get_nc():
    global _cached_nc
    if _cached_nc is None:
        _cached_nc = _build_device_kernel()
    return _cached_nc


def _silu(a):
    return a * (1.0 / (1.0 + np.exp(-a)))


def kernel(x, weight, bias, conv_state, seq_idx, conv_idx, state_ids,
           _run_opts=None):
    from concourse.bass_utils import run_bass_kernel_spmd

    x = np.asarray(x)
    weight = np.asarray(weight)
    bias = np.asarray(bias)
    conv_state = np.asarray(conv_state)
    seq_idx = np.asarray(seq_idx)
    conv_idx = np.asarray(conv_idx)
    state_ids = np.asarray(state_ids)

    x0 = x[0]                               # (T, D) f32
    w = weight[:, 0, :].astype(np.float32)  # (D, K)
    xT = np.ascontiguousarray(x0.T)         # (D, T)

    in_maps = []
    for c in range(NCORES):
        lo = c * DC
        w_core = w[lo:lo + DC]              # (DC, K)
        wsm = np.ascontiguousarray(
            w_core.reshape(G, 128, K).transpose(1, 0, 2))   # (128, G, K)
        bsm = np.ascontiguousarray(
            bias[lo:lo + DC].astype(np.float32).reshape(G, 128).T)  # (128, G)
        in_maps.append({
            "xt": np.ascontiguousarray(xT[lo:lo + DC]),
            "ws": wsm,
            "bs": bsm,
        })

    nc = _get_nc()
    run_opts = _run_opts or {}
    res = run_bass_kernel_spmd(nc, in_maps, core_ids=list(range(NCORES)),
                               **run_opts)

    outT = np.concatenate([r["yt"] for r in res.results], axis=0)  # (D, T)
    out = np.ascontiguousarray(outT.T)[None]                       # (1, T, D)

    # --- host fixup: first K-1 tokens of every sequence (exact recompute) ---
    starts = np.concatenate([[0], np.flatnonzero(np.diff(seq_idx) != 0) + 1])
    fix = (starts[:, None] + np.arange(HALO)[None]).ravel()
    fix = np.unique(fix[fix < T])
    if fix.size:
        acc = np.broadcast_to(bias.astype(np.float32), (fix.size, D)).copy()
        for j in range(K):
            s = K - 1 - j
            tm = fix - s
            tm_c = np.clip(tm, 0, T - 1)
            valid = (tm >= 0) & (seq_idx[tm_c] == seq_idx[fix])
            acc += np.where(valid[:, None], x0[tm_c], 0.0) * w[None, :, j]
        out[0, fix] = _silu(acc)

    # --- conv-state cache update (gather last-K rows, scatter into pool) ---
    new_conv_state = conv_state.copy()
    new_conv_state[state_ids] = np.transpose(x0[conv_idx], (0, 2, 1))

    if _run_opts is not None:
        return (out, new_conv_state), res
    return out, new_conv_state


# revision 24
# speedup vs baseline: 1.0199x; 1.0082x over previous
"""Causal depthwise conv1d (K=4) over packed ragged sequences + SiLU + conv-state
cache update, sharded channel-wise across 8 trn2 NeuronCores.

Strategy:
  - Channels (D=4096) sharded 512/core (tensor-parallel, per the module's tp
    logic). Host transposes x to channel-major (D, T) so each core DMAs
    contiguous rows; on-chip layout is [channels->partitions, tokens->free],
    so conv taps are free-dim offsets.
  - HBM reads need long per-partition rows (2KB rows ~250GB/s vs 32KB rows
    ~395GB/s with all 8 cores saturating device HBM), so x is loaded in
    [128, 8192] megatiles; writes are insensitive to chunk size, so each
    compute item stores its own output tile (keeps the pipeline
    fine-grained).
  - No single engine covers 4 fp32 taps under the HBM floor (~175us/core):
    fp32 PE matmul is 2-pass (~2.4cyc/col sustained) and fp32 DVE
    tensor-tensor ops run at 1 elem/lane/cyc. Token ranges are split:
      * PE tiles (512 tok): 4 diagonal-matrix matmuls accumulate taps in
        PSUM (per-channel scale = diagonal stationary); ScalarE
        silu(psum+bias).
      * DVE quads (2048/1536 tok): ScalarE seeds tap0 (Copy with per-channel
        scale), VectorE chains 3 scalar_tensor_tensor fused MACs in place,
        ScalarE silu.
    ~34 PE tiles vs ~23 quads per core balances PE (~170us) against DVE
    (~165us), matching the min-max assignment over (PE 1.24us, DVE 0.59us,
    ACT-seed 0.61us) per 512-token tap unit.
  - Sequence-boundary tokens (first 3 of each sequence, <= 27 rows) are
    recomputed exactly on the host; the conv-state cache update (1MB
    gather/scatter) is metadata-sized and also done on the host.
"""

import numpy as np

T = 16384
D = 4096
K = 4
NCORES = 8
DC = D // NCORES  # 512 channels per core
G = DC // 128     # 4 partition groups per core
HALO = K - 1      # 3

F_IO = 8192       # input megatile tokens (32KB rows)
F_PE = 512        # PE tile (one fp32 PSUM bank)
F_DVE = 2048      # DVE quad

# Work items per megatile, as (kind, local_offset, width):
#   'A' half: 4 PE tiles + 3 quads (the plain half)
#   'E' half: 5 PE tiles + 2 quads-2048 + 1 quad-1536 (PE-heavier)
_A = [("dve", 2048, 2048), ("dve", 4096, 2048), ("dve", 6144, 2048),
      ("pe", 0, F_PE), ("pe", 512, F_PE), ("pe", 1024, F_PE),
      ("pe", 1536, F_PE)]
_E = [("dve", 2048, 2048), ("dve", 4096, 2048), ("dve", 6144, 1536),
      ("pe", 0, F_PE), ("pe", 512, F_PE), ("pe", 1024, F_PE),
      ("pe", 1536, F_PE), ("pe", 7680, F_PE)]
# even groups: 8 PE tiles; odd groups: 9 PE tiles  -> 34 PE tiles/core
GROUP_PLANS = {0: (_A, _E), 1: (_A, _E), 2: (_A, _E), 3: (_A, _E)}

_cached_nc = None


def _build_device_kernel():
    import concourse.bacc as bacc
    import concourse.mybir as mybir
    from concourse.masks import make_identity
    from concourse.tile import TileContext

    f32 = mybir.dt.float32
    mult = mybir.AluOpType.mult
    add = mybir.AluOpType.add
    silu_fn = mybir.ActivationFunctionType.Silu
    copy_fn = mybir.ActivationFunctionType.Copy

    nc = bacc.Bacc("TRN2", target_bir_lowering=False, debug=False,
                   num_devices=NCORES)

    xt = nc.dram_tensor("xt", [DC, T], f32, kind="ExternalInput")
    ws = nc.dram_tensor("ws", [128, G, K], f32, kind="ExternalInput")
    bs = nc.dram_tensor("bs", [128, G], f32, kind="ExternalInput")
    yt = nc.dram_tensor("yt", [DC, T], f32, kind="ExternalOutput")

    with TileContext(nc) as tc:
        with (
            tc.tile_pool(name="const", bufs=1) as cpool,
            tc.tile_pool(name="xb", bufs=4) as xpool,
            tc.tile_pool(name="ype", bufs=8) as ype_pool,
            tc.tile_pool(name="acc", bufs=4) as acc_pool,
            tc.tile_pool(name="ps", bufs=8, space="PSUM") as ppool,
        ):
            ws_sb = cpool.tile([128, G, K], f32)
            nc.sync.dma_start(out=ws_sb[:], in_=ws[:])
            bs_sb = cpool.tile([128, G], f32)
            nc.sync.dma_start(out=bs_sb[:], in_=bs[:])
            ident = cpool.tile([128, 128], f32)
            make_identity(nc, ident[:])
            wd_sb = cpool.tile([128, G, K, 128], f32)
            for g in range(G):
                for j in range(K):
                    nc.vector.tensor_scalar_mul(
                        wd_sb[:, g, j, :], ident[:], ws_sb[:, g, j:j + 1])

            for g in range(G):
                rows = slice(g * 128, (g + 1) * 128)
                for half, plan in enumerate(GROUP_PLANS[g]):
                    t0 = half * F_IO
                    xb = xpool.tile([128, F_IO + HALO], f32)
                    if t0 == 0:
                        nc.gpsimd.memset(xb[:, 0:HALO], 0.0)
                        nc.sync.dma_start(out=xb[:, HALO:], in_=xt[rows, 0:F_IO])
                    else:
                        nc.sync.dma_start(
                            out=xb[:], in_=xt[rows, t0 - HALO:t0 + F_IO])

                    for kind, u, width in plan:
                        if kind == "pe":
                            ps = ppool.tile([128, F_PE], f32)
                            for j in range(K):
                                # tap j: out[c,t] += w[c,j]*x[c, t-(K-1-j)]
                                nc.tensor.matmul(
                                    ps[:], wd_sb[:, g, j, :],
                                    xb[:, u + j:u + j + F_PE],
                                    start=(j == 0), stop=(j == K - 1),
                                )
                            yp = ype_pool.tile([128, F_PE], f32)
                            nc.scalar.activation(
                                yp[:], ps[:], silu_fn,
                                bias=bs_sb[:, g:g + 1], scale=1.0)
                            nc.sync.dma_start(
                                out=yt[rows, t0 + u:t0 + u + F_PE], in_=yp[:])
                        else:
                            acc = acc_pool.tile([128, F_DVE], f32)
                            av = acc[:, 0:width]
                            nc.scalar.activation(
                                av, xb[:, u:u + width], copy_fn,
                                bias=0.0, scale=ws_sb[:, g, 0:1])
                            for j in (1, 2, 3):
                                nc.vector.scalar_tensor_tensor(
                                    out=av, in0=xb[:, u + j:u + j + width],
                                    scalar=ws_sb[:, g, j:j + 1], in1=av,
                                    op0=mult, op1=add)
                            nc.scalar.activation(
                                av, av, silu_fn,
                                bias=bs_sb[:, g:g + 1], scale=1.0)
                            nc.sync.dma_start(
                                out=yt[rows, t0 + u:t0 + u + width], in_=av)

    nc.compile()
    return nc


def _get_nc():
    global _cached_nc
    if _cached_nc is None:
        _cached_nc = _build_device_kernel()
    return _cached_nc


def _silu(a):
    return a * (1.0 / (1.0 + np.exp(-a)))


def kernel(x, weight, bias, conv_state, seq_idx, conv_idx, state_ids,
           _run_opts=None):
    from concourse.bass_utils import run_bass_kernel_spmd

    x = np.asarray(x)
    weight = np.asarray(weight)
    bias = np.asarray(bias)
    conv_state = np.asarray(conv_state)
    seq_idx = np.asarray(seq_idx)
    conv_idx = np.asarray(conv_idx)
    state_ids = np.asarray(state_ids)

    x0 = x[0]                               # (T, D) f32
    w = weight[:, 0, :].astype(np.float32)  # (D, K)
    xT = np.ascontiguousarray(x0.T)         # (D, T)

    in_maps = []
    for c in range(NCORES):
        lo = c * DC
        w_core = w[lo:lo + DC]              # (DC, K)
        wsm = np.ascontiguousarray(
            w_core.reshape(G, 128, K).transpose(1, 0, 2))   # (128, G, K)
        bsm = np.ascontiguousarray(
            bias[lo:lo + DC].astype(np.float32).reshape(G, 128).T)  # (128, G)
        in_maps.append({
            "xt": np.ascontiguousarray(xT[lo:lo + DC]),
            "ws": wsm,
            "bs": bsm,
        })

    nc = _get_nc()
    run_opts = _run_opts or {}
    res = run_bass_kernel_spmd(nc, in_maps, core_ids=list(range(NCORES)),
                               **run_opts)

    outT = np.concatenate([r["yt"] for r in res.results], axis=0)  # (D, T)
    out = np.ascontiguousarray(outT.T)[None]                       # (1, T, D)

    # --- host fixup: first K-1 tokens of every sequence (exact recompute) ---
    starts = np.concatenate([[0], np.flatnonzero(np.diff(seq_idx) != 0) + 1])
    fix = (starts[:, None] + np.arange(HALO)[None]).ravel()
    fix = np.unique(fix[fix < T])
    if fix.size:
        acc = np.broadcast_to(bias.astype(np.float32), (fix.size, D)).copy()
        for j in range(K):
            s = K - 1 - j
            tm = fix - s
            tm_c = np.clip(tm, 0, T - 1)
            valid = (tm >= 0) & (seq_idx[tm_c] == seq_idx[fix])
            acc += np.where(valid[:, None], x0[tm_c], 0.0) * w[None, :, j]
        out[0, fix] = _silu(acc)

    # --- conv-state cache update (gather last-K rows, scatter into pool) ---
    new_conv_state = conv_state.copy()
    new_conv_state[state_ids] = np.transpose(x0[conv_idx], (0, 2, 1))

    if _run_opts is not None:
        return (out, new_conv_state), res
    return out, new_conv_state


# revision 26
# speedup vs baseline: 1.0369x; 1.0167x over previous
"""Causal depthwise conv1d (K=4) over packed ragged sequences + SiLU + conv-state
cache update, sharded channel-wise across 8 trn2 NeuronCores.

Strategy:
  - Channels (D=4096) sharded 512/core (tensor-parallel, per the module's tp
    logic). Host transposes x to channel-major (D, T) so each core DMAs
    contiguous rows; on-chip layout is [channels->partitions, tokens->free],
    so conv taps are free-dim offsets.
  - HBM reads need long per-partition rows (2KB rows ~250GB/s vs 32KB rows
    ~395GB/s with all 8 cores saturating device HBM), so x is loaded in
    [128, 8192] megatiles; writes are insensitive to chunk size, so each
    compute item stores its own output tile (keeps the pipeline
    fine-grained).
  - No single engine covers 4 fp32 taps under the HBM floor (~175us/core):
    fp32 PE matmul is 2-pass (~2.4cyc/col sustained) and fp32 DVE
    tensor-tensor ops run at 1 elem/lane/cyc. Token ranges are split:
      * PE tiles (512 tok): 4 diagonal-matrix matmuls accumulate taps in
        PSUM (per-channel scale = diagonal stationary); ScalarE
        silu(psum+bias).
      * DVE quads (2048/1536 tok): ScalarE seeds tap0 (Copy with per-channel
        scale), VectorE chains 3 scalar_tensor_tensor fused MACs in place,
        ScalarE silu.
    ~34 PE tiles vs ~23 quads per core balances PE (~170us) against DVE
    (~165us), matching the min-max assignment over (PE 1.24us, DVE 0.59us,
    ACT-seed 0.61us) per 512-token tap unit.
  - Sequence-boundary tokens (first 3 of each sequence, <= 27 rows) are
    recomputed exactly on the host; the conv-state cache update (1MB
    gather/scatter) is metadata-sized and also done on the host.
"""

import numpy as np

T = 16384
D = 4096
K = 4
NCORES = 8
DC = D // NCORES  # 512 channels per core
G = DC // 128     # 4 partition groups per core
HALO = K - 1      # 3

F_IO = 8192       # input megatile tokens (32KB rows)
F_PE = 512        # PE tile (one fp32 PSUM bank)
F_DVE = 2048      # DVE quad

# Work items per megatile, as (kind, local_offset, width):
#   'A' half: 4 PE tiles + 3 quads (the plain half)
#   'E' half: 5 PE tiles + 2 quads-2048 + 1 quad-1536 (PE-heavier)
_A = [("dve", 2048, 2048), ("dve", 4096, 2048), ("dve", 6144, 2048),
      ("pe", 0, F_PE), ("pe", 512, F_PE), ("pe", 1024, F_PE),
      ("pe", 1536, F_PE)]
_E = [("dve", 2048, 2048), ("dve", 4096, 2048), ("dve", 6144, 1536),
      ("pe", 0, F_PE), ("pe", 512, F_PE), ("pe", 1024, F_PE),
      ("pe", 1536, F_PE), ("pe", 7680, F_PE)]
# even groups: 8 PE tiles; odd groups: 9 PE tiles  -> 34 PE tiles/core
GROUP_PLANS = {0: (_A, _E), 1: (_A, _E), 2: (_A, _E), 3: (_A, _E)}

_cached_nc = None


def _build_device_kernel():
    import concourse.bacc as bacc
    import concourse.mybir as mybir
    from concourse.masks import make_identity
    from concourse.tile import TileContext

    f32 = mybir.dt.float32
    mult = mybir.AluOpType.mult
    add = mybir.AluOpType.add
    silu_fn = mybir.ActivationFunctionType.Silu
    copy_fn = mybir.ActivationFunctionType.Copy

    nc = bacc.Bacc("TRN2", target_bir_lowering=False, debug=False,
                   num_devices=NCORES)

    xt = nc.dram_tensor("xt", [DC, T], f32, kind="ExternalInput")
    ws = nc.dram_tensor("ws", [128, G, K], f32, kind="ExternalInput")
    bs = nc.dram_tensor("bs", [128, G], f32, kind="ExternalInput")
    yt = nc.dram_tensor("yt", [DC, T], f32, kind="ExternalOutput")

    with TileContext(nc) as tc:
        with (
            tc.tile_pool(name="const", bufs=1) as cpool,
            tc.tile_pool(name="xb", bufs=4) as xpool,
            tc.tile_pool(name="ype", bufs=8) as ype_pool,
            tc.tile_pool(name="acc", bufs=4) as acc_pool,
            tc.tile_pool(name="ps", bufs=8, space="PSUM") as ppool,
        ):
            ws_sb = cpool.tile([128, G, K], f32)
            nc.sync.dma_start(out=ws_sb[:], in_=ws[:])
            bs_sb = cpool.tile([128, G], f32)
            nc.sync.dma_start(out=bs_sb[:], in_=bs[:])
            ident = cpool.tile([128, 128], f32)
            make_identity(nc, ident[:])
            wd_sb = cpool.tile([128, G, K, 128], f32)
            for g in range(G):
                for j in range(K):
                    nc.vector.tensor_scalar_mul(
                        wd_sb[:, g, j, :], ident[:], ws_sb[:, g, j:j + 1])

            for g in range(G):
                rows = slice(g * 128, (g + 1) * 128)
                for half, plan in enumerate(GROUP_PLANS[g]):
                    t0 = half * F_IO
                    xb = xpool.tile([128, F_IO + HALO], f32)
                    if t0 == 0:
                        nc.gpsimd.memset(xb[:, 0:HALO], 0.0)
                        nc.sync.dma_start(out=xb[:, HALO:], in_=xt[rows, 0:F_IO])
                    else:
                        nc.sync.dma_start(
                            out=xb[:], in_=xt[rows, t0 - HALO:t0 + F_IO])

                    for kind, u, width in plan:
                        if kind == "pe":
                            ps = ppool.tile([128, F_PE], f32)
                            for j in range(K):
                                # tap j: out[c,t] += w[c,j]*x[c, t-(K-1-j)]
                                nc.tensor.matmul(
                                    ps[:], wd_sb[:, g, j, :],
                                    xb[:, u + j:u + j + F_PE],
                                    start=(j == 0), stop=(j == K - 1),
                                )
                            yp = ype_pool.tile([128, F_PE], f32)
                            nc.scalar.activation(
                                yp[:], ps[:], silu_fn,
                                bias=bs_sb[:, g:g + 1], scale=1.0)
                            nc.sync.dma_start(
                                out=yt[rows, t0 + u:t0 + u + F_PE], in_=yp[:])
                        else:
                            acc = acc_pool.tile([128, F_DVE], f32)
                            av = acc[:, 0:width]
                            nc.scalar.activation(
                                av, xb[:, u:u + width], copy_fn,
                                bias=0.0, scale=ws_sb[:, g, 0:1])
                            for j in (1, 2, 3):
                                nc.vector.scalar_tensor_tensor(
                                    out=av, in0=xb[:, u + j:u + j + width],
                                    scalar=ws_sb[:, g, j:j + 1], in1=av,
                                    op0=mult, op1=add)
                            nc.scalar.activation(
                                av, av, silu_fn,
                                bias=bs_sb[:, g:g + 1], scale=1.0)
                            nc.sync.dma_start(
                                out=yt[rows, t0 + u:t0 + u + width], in_=av)

    nc.compile()
    return nc


def _get_nc():
    global _cached_nc
    if _cached_nc is None:
        _cached_nc = _build_device_kernel()
    return _cached_nc


def _silu(a):
    return a * (1.0 / (1.0 + np.exp(-a)))


def kernel(x, weight, bias, conv_state, seq_idx, conv_idx, state_ids,
           _run_opts=None):
    from concourse.bass_utils import run_bass_kernel_spmd

    x = np.asarray(x)
    weight = np.asarray(weight)
    bias = np.asarray(bias)
    conv_state = np.asarray(conv_state)
    seq_idx = np.asarray(seq_idx)
    conv_idx = np.asarray(conv_idx)
    state_ids = np.asarray(state_ids)

    x0 = x[0]                               # (T, D) f32
    w = weight[:, 0, :].astype(np.float32)  # (D, K)
    xT = np.ascontiguousarray(x0.T)         # (D, T)

    in_maps = []
    for c in range(NCORES):
        lo = c * DC
        w_core = w[lo:lo + DC]              # (DC, K)
        wsm = np.ascontiguousarray(
            w_core.reshape(G, 128, K).transpose(1, 0, 2))   # (128, G, K)
        bsm = np.ascontiguousarray(
            bias[lo:lo + DC].astype(np.float32).reshape(G, 128).T)  # (128, G)
        in_maps.append({
            "xt": np.ascontiguousarray(xT[lo:lo + DC]),
            "ws": wsm,
            "bs": bsm,
        })

    nc = _get_nc()
    run_opts = _run_opts or {}
    res = run_bass_kernel_spmd(nc, in_maps, core_ids=list(range(NCORES)),
                               **run_opts)

    outT = np.concatenate([r["yt"] for r in res.results], axis=0)  # (D, T)
    out = np.ascontiguousarray(outT.T)[None]                       # (1, T, D)

    # --- host fixup: first K-1 tokens of every sequence (exact recompute) ---
    starts = np.concatenate([[0], np.flatnonzero(np.diff(seq_idx) != 0) + 1])
    fix = (starts[:, None] + np.arange(HALO)[None]).ravel()
    fix = np.unique(fix[fix < T])
    if fix.size:
        acc = np.broadcast_to(bias.astype(np.float32), (fix.size, D)).copy()
        for j in range(K):
            s = K - 1 - j
            tm = fix - s
            tm_c = np.clip(tm, 0, T - 1)
            valid = (tm >= 0) & (seq_idx[tm_c] == seq_idx[fix])
            acc += np.where(valid[:, None], x0[tm_c], 0.0) * w[None, :, j]
        out[0, fix] = _silu(acc)

    # --- conv-state cache update (gather last-K rows, scatter into pool) ---
    new_conv_state = conv_state.copy()
    new_conv_state[state_ids] = np.transpose(x0[conv_idx], (0, 2, 1))

    if _run_opts is not None:
        return (out, new_conv_state), res
    return out, new_conv_state
